# revision 45
# baseline (speedup 1.0000x reference)
"""Multi-head attention (B=2, S=2048, D=768, H=12) on 8 TRN2 NeuronCores.

Sharding: core c -> batch b = c//4, head-group g = c%4 (3 heads of 64 each).
Each core computes q/k/v projections for its 3 heads, masked softmax
attention, and a partial output projection against its 192 columns of Wo.
Host sums the 4 partial outputs per batch element (fp16 partials, fp32 sum).

Perf notes (v2):
  - fp16 everywhere on-device (same cost as bf16, more mantissa).
  - PE is HAM-clock-gated (1.2 GHz cold, 2.4 GHz after ~3.4us of sustained
    work): warm-up matmuls run during the initial DMA wait and the whole
    kernel is emitted as one gap-free tensor stream.
  - v is projected directly into [seq, d] layout (x-stationary matmuls) so
    no PE transposes are needed.
  - Attention is a 1-step-lagged pipeline over (n-block, k-group) steps:
    energy matmuls for group g run while exp/mask of g-1 and attV of g-1
    run on scalar/vector, keeping all engines busy.
  - Softmax denominators: ones-column in the v tiles -> row 64 of the attV
    PSUM; reciprocal via the fast custom-DVE approx on [1,512] (the plain
    reciprocal costs 3.3us), broadcast on gpsimd.
  - Output projection packs heads 0+1 into one K=128 matmul (onA) plus a
    K=64 accumulate (onB), interleaved one chain per pipeline step.
"""

import os
import sys

sys.path.insert(0, "/opt/trn_rl_repo")

from contextlib import ExitStack

import numpy as np

import concourse.bass as bass
import concourse.mybir as mybir
import concourse.tile as tile
from concourse import bacc
from concourse.bass import ds
from concourse.bass_utils import run_bass_kernel_spmd

F32 = mybir.dt.float32
F16 = mybir.dt.float16

SEQ = 2048
D = 768
HD = 64
GD = 192          # head-group width = 3 heads * 64
QB = 512          # q-block (free dim of E^T matmuls)
NQB = SEQ // QB   # 4
KT = SEQ // 128   # 16 k-tiles
NG = KT // 2      # 8 k-groups of 2 tiles per q-block
SCALE = float(1.0 / np.sqrt(np.float32(D)))

_CACHE = {}


def _install_profile_hook():
    """The image's antenv lacks axon_hooks; synthesize it so
    run_bass_kernel_spmd(trace=True) can reach the NTFF profiler in
    libaxon_pjrt.so (same ctypes shim trn_agent_boot uses)."""
    import types

    if "antenv.axon_hooks" in sys.modules:
        return
    sys.path.insert(0, "/root/.axon_site")
    try:
        from trn_agent_boot.trn_boot import _ntff_profile_via_ctypes
        hook = _ntff_profile_via_ctypes("/opt/axon/libaxon_pjrt.so")
    except Exception:
        hook = None
    import concourse.bass_utils as _bu

    _bu.upload_artifacts = lambda tmpdir: tmpdir  # no artifact bucket here
    mod = types.ModuleType("antenv.axon_hooks")
    mod.get_axon_ntff_profile_hook = lambda: hook
    mod.set_axon_ntff_profile_hook = lambda h: None
    sys.modules["antenv.axon_hooks"] = mod


def _build():
    nc = bacc.Bacc(None)

    xqT = nc.declare_dram_parameter("xqT", [D, SEQ], F16, isOutput=False)
    xkT = nc.declare_dram_parameter("xkT", [D, SEQ], F16, isOutput=False)
    xvT = nc.declare_dram_parameter("xvT", [D, SEQ], F16, isOutput=False)
    wqT = nc.declare_dram_parameter("wqT", [D, GD], F16, isOutput=False)
    wkT = nc.declare_dram_parameter("wkT", [D, GD], F16, isOutput=False)
    wvT = nc.declare_dram_parameter("wvT", [D, GD], F16, isOutput=False)
    woT = nc.declare_dram_parameter("woT", [GD, D], F16, isOutput=False)
    maskT = nc.declare_dram_parameter("maskT", [SEQ, SEQ], F16, isOutput=False)
    out = nc.declare_dram_parameter("out", [SEQ, D], F16, isOutput=True)
    debug = bool(int(os.environ.get("KERNEL_DEBUG", "0")))
    if debug:
        dbg = {
            nm: nc.declare_dram_parameter(f"dbg_{nm}", shp, F16, isOutput=True)
            for nm, shp in (
                ("qA", [128, SEQ]), ("kA", [128, SEQ]),
                ("qB", [64, SEQ]), ("kB", [64, SEQ]),
                ("onA", [128, SEQ]), ("onB", [64, SEQ]),
                ("vaug", [128, KT * 3 * (HD + 1)]),
            )
        }

    with tile.TileContext(nc) as tc, ExitStack() as ctx:
        Exp = mybir.ActivationFunctionType.Exp

        # ---- persistent tiles --------------------------------------------
        pp = ctx.enter_context(tc.tile_pool(name="persist", bufs=1))
        qA = pp.tile([128, SEQ], F16, tag="qA")   # heads 0 (p0-63) / 1 (p64-127)
        qB = pp.tile([64, SEQ], F16, tag="qB")    # head 2
        kA = pp.tile([128, SEQ], F16, tag="kA")
        kB = pp.tile([64, SEQ], F16, tag="kB")
        # v in [k-seq, d+1] layout per head; col 64 = ones (softmax denom).
        # Per-head 3D tiles: a 4D [128,KT,3,65] tile sliced [:,m,h,:] loads
        # the PE stationary with misordered columns (observed on HW).
        vaug = [pp.tile([128, KT, HD + 1], F16, tag=f"vaug{h}",
                        name=f"vaug{h}") for h in range(3)]
        onA = pp.tile([128, SEQ], F16, tag="onA")  # normalized out, heads 0/1
        onB = pp.tile([64, SEQ], F16, tag="onB")   # head 2
        woA = pp.tile([128, D], F16, tag="woA")
        woB = pp.tile([64, D], F16, tag="woB")
        w_sb = {n: [pp.tile([128, GD], F16, tag=f"w{n}{k}", name=f"w_{n}_{k}")
                    for k in range(6)] for n in ("q", "k", "v")}
        zt = pp.tile([128, QB], F16, tag="zt")    # zeros for PE warm-up

        nc.vector.memset(zt[:], 0.0)
        for h in range(3):
            nc.vector.memset(vaug[h][:, :, HD : HD + 1], 1.0)

        # weight DMAs issue on the scalar queue interleaved with its share of
        # x tiles (emitted inside issue_x below); wo lands late, on sync
        w_dma_todo = [(w_sb[name][k], wT[ds(k * 128, 128), :])
                      for name, wT in (("k", wkT), ("q", wqT), ("v", wvT))
                      for k in range(6)]

        xp = ctx.enter_context(tc.tile_pool(name="xp", bufs=12))
        mp = ctx.enter_context(tc.tile_pool(name="mp", bufs=16))
        # 6 P bufs = 2 full blocks of separation, so a new block's exp never
        # lands in a slot whose attV readers haven't been emitted yet
        pp2 = ctx.enter_context(tc.tile_pool(name="P", bufs=6))
        rp = ctx.enter_context(tc.tile_pool(name="rp", bufs=2))
        op = ctx.enter_context(tc.tile_pool(name="op", bufs=2))

        maskR = maskT.rearrange("(ko ki) q -> ki ko q", ki=128)
        masks = {}

        def issue_mask(n, j0=0, j1=8):
            tiles = masks.setdefault(n, [])
            for j in range(j0, j1):
                t = mp.tile([128, 2, QB], F16, tag="mask", name=f"mask{n}_{j}")
                eng = nc.gpsimd if j % 2 == 0 else nc.sync
                eng.dma_start(t[:], maskR[:, ds(j * 2, 2), ds(n * QB, QB)])
                tiles.append(t)

        # ---- phase 1: projections ----------------------------------------
        with tc.tile_pool(name="pj_ps", bufs=2, space="PSUM") as pj_ps, \
             tc.tile_pool(name="pv_ps", bufs=2, space="PSUM") as pv_ps:

            # PE warm-up: junk matmuls on zeros while the x DMAs stream in.
            # Keeps the HAM clock gate at 8/8 so the first real chains run
            # at 2.4 GHz (~12us of cover until xk lands).
            wps = pj_ps.tile([128, QB], F32, tag="warm")
            for _ in range(36):
                nc.tensor.matmul(wps[:], lhsT=zt[:, 0:128], rhs=zt[:],
                                 start=True, stop=True)

            # consumption order is k -> q -> v: energy needs the full kA at
            # attention step 0, q blocks and v tiles follow. x stripes over
            # all three DMA-capable queues (each queue serializes its
            # transfers at ~160GB/s); the w chunks ride along on scalar.
            x_engines = [nc.gpsimd, nc.sync, nc.scalar]

            def issue_x(name, xT):
                ts = []
                for nb2 in range(2):
                    for k in range(6):
                        xt = xp.tile([128, 1024], F16, tag="x",
                                     name=f"x_{name}_{nb2}_{k}")
                        eng = x_engines[(nb2 * 6 + k) % 3]
                        eng.dma_start(
                            xt[:], xT[ds(k * 128, 128), ds(nb2 * 1024, 1024)]
                        )
                        ts.append(xt)
                for _ in range(6):
                    if w_dma_todo:
                        wt, src = w_dma_todo.pop(0)
                        nc.scalar.dma_start(wt[:], src)
                return ts

            dests = {"q": (qA, qB), "k": (kA, kB)}
            xk_t = issue_x("k", xkT)
            for name in ("k", "q"):
                xs = xk_t if name == "k" else xq_t
                for nb2 in range(2):
                    for half in range(2):
                        n = nb2 * 2 + half
                        for mt in range(2):
                            # keep-warm junk while the chain waits on x DMAs
                            for _ in range(4):
                                nc.tensor.matmul(wps[:, 0:128],
                                                 lhsT=zt[:, 0:128],
                                                 rhs=zt[:, 0:128], start=True,
                                                 stop=True)
                            mw = 128 if mt == 0 else 64
                            ps = pj_ps.tile([128, QB], F32, tag="pj")
                            for k in range(6):
                                nc.tensor.matmul(
                                    ps[0:mw, :],
                                    lhsT=w_sb[name][k][:, ds(mt * 128, mw)],
                                    rhs=xs[nb2 * 6 + k][:, ds(half * QB, QB)],
                                    start=(k == 0),
                                    stop=(k == 5),
                                )
                            dst = dests[name][mt]
                            nc.scalar.copy(
                                dst[0:mw, ds(n * QB, QB)], ps[0:mw, :]
                            )
                if name == "k":
                    xq_t = issue_x("q", xqT)
                else:
                    xv_t = issue_x("v", xvT)
                    issue_mask(0)   # after all x: masks only needed ~45us in
                    issue_mask(1)
                    nc.sync.dma_start(woA[:], woT[0:128, :])
                    nc.sync.dma_start(woB[:], woT[128:GD, :])
            for nb2 in range(2):
                for sb in range(8):
                    kt = nb2 * 8 + sb
                    pv = pv_ps.tile([128, 3, HD], F32, tag="pv")
                    for k in range(6):
                        nc.tensor.matmul(
                            pv[:, :, :],
                            lhsT=xv_t[nb2 * 6 + k][:, ds(sb * 128, 128)],
                            rhs=w_sb["v"][k][:].rearrange(
                                "p (h d) -> p h d", h=3
                            ),
                            start=(k == 0),
                            stop=(k == 5),
                        )
                    for h in range(3):
                        nc.vector.tensor_copy(
                            vaug[h][:, kt, 0:HD], pv[:, h, :]
                        )

        # ---- phase 2: attention + output projection, one pipeline -------
        q_of = (qA, qA, qB)
        k_of = (kA, kA, kB)
        pbase = (0, 64, 0)
        P = {}
        OU = {}
        pending = []   # deferred output-projection chains (n, j)

        with tc.tile_pool(name="e_ps", bufs=2, space="PSUM") as e_ps, \
             tc.tile_pool(name="ou_ps", bufs=3, space="PSUM") as ou_ps, \
             tc.tile_pool(name="f_ps", bufs=1, space="PSUM") as f_ps:

            def energy(n, g, h):
                e = e_ps.tile([128, 2, QB], F32, tag="e")
                p0 = pbase[h]
                for mm in range(2):
                    m = 2 * g + mm
                    nc.tensor.matmul(
                        e[:, mm, :],
                        lhsT=k_of[h][p0 : p0 + 64, ds(m * 128, 128)],
                        rhs=q_of[h][p0 : p0 + 64, ds(n * QB, QB)],
                        start=True,
                        stop=True,
                    )
                sl = ds(2 * g, 2)
                nc.scalar.activation(P[(n, h)][:, sl, :], e[:, :, :], Exp,
                                     scale=SCALE)
                nc.vector.tensor_mul(P[(n, h)][:, sl, :], P[(n, h)][:, sl, :],
                                     masks[n][g][:, :, :])

            def attv_seg(n, g):
                # head 2's chain starts one group late (its ou bank frees one
                # step later than heads 0/1) and catches up at g == 1
                if g == 0:
                    plan = [(0, (0, 1)), (1, (0, 1))]
                    for h in (0, 1):
                        OU[(n, h)] = ou_ps.tile([HD + 1, QB], F32, tag="ou",
                                                name=f"ou{n}_{h}")
                elif g == 1:
                    OU[(n, 2)] = ou_ps.tile([HD + 1, QB], F32, tag="ou",
                                            name=f"ou{n}_2")
                    plan = [(0, (2, 3)), (1, (2, 3)), (2, (0, 1, 2, 3))]
                else:
                    plan = [(h, (2 * g, 2 * g + 1)) for h in range(3)]
                for h, mms in plan:
                    ou = OU[(n, h)]
                    for mm in mms:
                        nc.tensor.matmul(
                            ou[:],
                            lhsT=vaug[h][:, mm, :],
                            rhs=P[(n, h)][:, mm, :],
                            start=(mm == 0),
                            stop=(mm == KT - 1),
                        )

            def norm_h(n, h):
                ou = OU.pop((n, h))
                # native tensor_copy remaps partition 64 -> 0; the custom
                # DVE recip op ignores partition offsets on its operands
                dsb = rp.tile([1, QB], F32, tag="dsb")
                nc.vector.tensor_copy(dsb[:], ou[HD : HD + 1, :])
                r1 = rp.tile([1, QB], F32, tag="r1")
                nc.vector.reciprocal_approx_fast(r1[:], dsb[:])
                rb = rp.tile([HD, QB], F32, tag="rb")
                nc.gpsimd.partition_broadcast(rb[:], r1[:])
                if h == 2:
                    dst = onB[0:HD, ds(n * QB, QB)]
                else:
                    dst = onA[pbase[h] : pbase[h] + HD, ds(n * QB, QB)]
                nc.vector.tensor_mul(dst, ou[0:HD, :], rb[:])

            osb = {}

            def outproj_chain(n, j, pool=None, tail=False):
                mq = n * 4 + j // 2
                half = j % 2
                c0 = half * 384
                if half == 0:
                    osb[mq] = op.tile([128, D], F16, tag="o", name=f"o{mq}")
                p = pool or f_ps
                f = p.tile([128, 384], F32, tag="ou" if p is ou_ps else "f")
                nc.tensor.matmul(f[:], lhsT=onA[:, ds(mq * 128, 128)],
                                 rhs=woA[:, ds(c0, 384)], start=True, stop=False)
                nc.tensor.matmul(f[:], lhsT=onB[0:HD, ds(mq * 128, 128)],
                                 rhs=woB[0:HD, ds(c0, 384)], start=False,
                                 stop=True)
                # in the tail, alternate copies over scalar+vector so neither
                # queue serializes all 16 of them; in-block, vector has slack
                if tail and half == 0:
                    nc.scalar.copy(osb[mq][:, ds(c0, 384)], f[:])
                else:
                    nc.vector.tensor_copy(osb[mq][:, ds(c0, 384)], f[:])
                if half == 1:
                    o = osb.pop(mq)
                    if tail:
                        # last block: 4-way split across 2 queues to shrink
                        # the final-store latency
                        for i in range(4):
                            eng = nc.sync if i % 2 == 0 else nc.scalar
                            eng.dma_start(
                                out[ds(mq * 128, 128), ds(i * 192, 192)],
                                o[:, ds(i * 192, 192)],
                            )
                    else:
                        nc.sync.dma_start(out[ds(mq * 128, 128), 0:384],
                                          o[:, 0:384])
                        nc.sync.dma_start(out[ds(mq * 128, 128), 384:D],
                                          o[:, 384:D])

            # attV lags energy by THREE steps, and the previous block's
            # normalization is spread one head per step (emitted before the
            # attV that recycles its ou bank) so the vector queue never
            # bunches at block boundaries
            LAG = 3
            TOT = NQB * NG
            for s in range(TOT + LAG + 3):
                n, g = s // NG, s % NG
                if s < TOT:
                    if g == 0:
                        for h in range(3):
                            P[(n, h)] = pp2.tile([128, KT, QB], F16, tag="P",
                                                 name=f"P{n}_{h}")
                    energy(n, g, 0)
                    energy(n, g, 1)
                t = s - LAG
                if t >= 0:
                    if t % NG == 0 and t // NG >= 1:
                        norm_h(t // NG - 1, 1)
                    if t % NG == 1 and t // NG >= 1:
                        norm_h(t // NG - 1, 2)
                        pending.extend((t // NG - 1, j) for j in range(8))
                    if t < TOT:
                        attv_seg(t // NG, t % NG)
                    if t % NG == NG - 1:
                        norm_h(t // NG, 0)
                if s < TOT:
                    energy(n, g, 2)
                if pending:
                    outproj_chain(*pending.pop(0))
                # mask prefetch for block n+2, only for groups whose block-n
                # readers (the mask muls) have already been emitted
                if s < TOT and n + 2 < NQB:
                    if g == 5:
                        issue_mask(n + 2, 0, 5)
                    elif g == 7:
                        issue_mask(n + 2, 5, 8)
            # tail: remaining output projection; ou banks are free now, so
            # rotate f tiles through them for a deeper PSUM pipeline
            tail_pools = [f_ps, ou_ps, ou_ps, ou_ps]
            ti = 0
            while pending:
                outproj_chain(*pending.pop(0), pool=tail_pools[ti % 4],
                              tail=True)
                ti += 1

        if debug:
            for nm, t in (("qA", qA), ("kA", kA), ("qB", qB), ("kB", kB),
                          ("onA", onA), ("onB", onB)):
                nc.sync.dma_start(dbg[nm][0 : t.shape[0], :], t[:])
            nc.sync.dma_start(
                dbg["vaug"][:, 0 : KT * (HD + 1)],
                vaug[0][:].rearrange("p a c -> p (a c)"),
            )

    nc.compile()
    return nc


def kernel(Q, K, V, mask, Wq, Wk, Wv, Wo):
    if "nc" not in _CACHE:
        _CACHE["nc"] = _build()
    nc = _CACHE["nc"]

    maskT_f16 = np.ascontiguousarray((mask[0, 0].T != 0).astype(np.float16))
    in_maps = []
    for c in range(8):
        b, g = c // 4, c % 4
        sl = slice(g * GD, (g + 1) * GD)
        in_maps.append(
            {
                "xqT": np.ascontiguousarray(Q[b].T.astype(np.float16)),
                "xkT": np.ascontiguousarray(K[b].T.astype(np.float16)),
                "xvT": np.ascontiguousarray(V[b].T.astype(np.float16)),
                "wqT": np.ascontiguousarray(Wq[sl, :].T.astype(np.float16)),
                "wkT": np.ascontiguousarray(Wk[sl, :].T.astype(np.float16)),
                "wvT": np.ascontiguousarray(Wv[sl, :].T.astype(np.float16)),
                "woT": np.ascontiguousarray(Wo[:, sl].T.astype(np.float16)),
                "maskT": maskT_f16,
            }
        )

    _install_profile_hook()
    res = run_bass_kernel_spmd(
        nc,
        in_maps,
        core_ids=list(range(8)),
        trace=bool(int(os.environ.get("KERNEL_PROFILE", "0"))),
    )
    _CACHE["last_exec_ns"] = res.exec_time_ns

    out = np.zeros((2, SEQ, D), dtype=np.float32)
    for c in range(8):
        out[c // 4] += res.results[c]["out"].astype(np.float32)
    return out


# revision 48
# speedup vs baseline: 1.0232x; 1.0232x over previous
"""Multi-head attention (B=2, S=2048, D=768, H=12) on 8 TRN2 NeuronCores.

Sharding: core c -> batch b = c//4, head-group g = c%4 (3 heads of 64 each).
Each core computes q/k/v projections for its 3 heads, masked softmax
attention, and a partial output projection against its 192 columns of Wo.
Host sums the 4 partial outputs per batch element (fp16 partials, fp32 sum).

Perf notes (v2):
  - fp16 everywhere on-device (same cost as bf16, more mantissa).
  - PE is HAM-clock-gated (1.2 GHz cold, 2.4 GHz after ~3.4us of sustained
    work): warm-up matmuls run during the initial DMA wait and the whole
    kernel is emitted as one gap-free tensor stream.
  - v is projected directly into [seq, d] layout (x-stationary matmuls) so
    no PE transposes are needed.
  - Attention is a 1-step-lagged pipeline over (n-block, k-group) steps:
    energy matmuls for group g run while exp/mask of g-1 and attV of g-1
    run on scalar/vector, keeping all engines busy.
  - Softmax denominators: ones-column in the v tiles -> row 64 of the attV
    PSUM; reciprocal via the fast custom-DVE approx on [1,512] (the plain
    reciprocal costs 3.3us), broadcast on gpsimd.
  - Output projection packs heads 0+1 into one K=128 matmul (onA) plus a
    K=64 accumulate (onB), interleaved one chain per pipeline step.
"""

import os
import sys

sys.path.insert(0, "/opt/trn_rl_repo")

from contextlib import ExitStack

import numpy as np

import concourse.bass as bass
import concourse.mybir as mybir
import concourse.tile as tile
from concourse import bacc
from concourse.bass import ds
from concourse.bass_utils import run_bass_kernel_spmd

F32 = mybir.dt.float32
F16 = mybir.dt.float16

SEQ = 2048
D = 768
HD = 64
GD = 192          # head-group width = 3 heads * 64
QB = 512          # q-block (free dim of E^T matmuls)
NQB = SEQ // QB   # 4
KT = SEQ // 128   # 16 k-tiles
NG = KT // 2      # 8 k-groups of 2 tiles per q-block
SCALE = float(1.0 / np.sqrt(np.float32(D)))

_CACHE = {}


def _install_profile_hook():
    """The image's antenv lacks axon_hooks; synthesize it so
    run_bass_kernel_spmd(trace=True) can reach the NTFF profiler in
    libaxon_pjrt.so (same ctypes shim trn_agent_boot uses)."""
    import types

    if "antenv.axon_hooks" in sys.modules:
        return
    sys.path.insert(0, "/root/.axon_site")
    try:
        from trn_agent_boot.trn_boot import _ntff_profile_via_ctypes
        hook = _ntff_profile_via_ctypes("/opt/axon/libaxon_pjrt.so")
    except Exception:
        hook = None
    import concourse.bass_utils as _bu

    _bu.upload_artifacts = lambda tmpdir: tmpdir  # no artifact bucket here
    mod = types.ModuleType("antenv.axon_hooks")
    mod.get_axon_ntff_profile_hook = lambda: hook
    mod.set_axon_ntff_profile_hook = lambda h: None
    sys.modules["antenv.axon_hooks"] = mod


def _build():
    nc = bacc.Bacc(None)

    xqT = nc.declare_dram_parameter("xqT", [D, SEQ], F16, isOutput=False)
    xkT = nc.declare_dram_parameter("xkT", [D, SEQ], F16, isOutput=False)
    xvT = nc.declare_dram_parameter("xvT", [D, SEQ], F16, isOutput=False)
    wqT = nc.declare_dram_parameter("wqT", [D, GD], F16, isOutput=False)
    wkT = nc.declare_dram_parameter("wkT", [D, GD], F16, isOutput=False)
    wvT = nc.declare_dram_parameter("wvT", [D, GD], F16, isOutput=False)
    woT = nc.declare_dram_parameter("woT", [GD, D], F16, isOutput=False)
    maskT = nc.declare_dram_parameter("maskT", [SEQ, SEQ], F16, isOutput=False)
    out = nc.declare_dram_parameter("out", [SEQ, D], F16, isOutput=True)
    debug = bool(int(os.environ.get("KERNEL_DEBUG", "0")))
    if debug:
        dbg = {
            nm: nc.declare_dram_parameter(f"dbg_{nm}", shp, F16, isOutput=True)
            for nm, shp in (
                ("qA", [128, SEQ]), ("kA", [128, SEQ]),
                ("qB", [64, SEQ]), ("kB", [64, SEQ]),
                ("onA", [128, SEQ]), ("onB", [64, SEQ]),
                ("vaug", [128, KT * 3 * (HD + 1)]),
            )
        }

    with tile.TileContext(nc) as tc, ExitStack() as ctx:
        Exp = mybir.ActivationFunctionType.Exp

        # ---- persistent tiles --------------------------------------------
        pp = ctx.enter_context(tc.tile_pool(name="persist", bufs=1))
        qA = pp.tile([128, SEQ], F16, tag="qA")   # heads 0 (p0-63) / 1 (p64-127)
        qB = pp.tile([64, SEQ], F16, tag="qB")    # head 2
        kA = pp.tile([128, SEQ], F16, tag="kA")
        kB = pp.tile([64, SEQ], F16, tag="kB")
        # v in [k-seq, d+1] layout per head; col 64 = ones (softmax denom).
        # Per-head 3D tiles: a 4D [128,KT,3,65] tile sliced [:,m,h,:] loads
        # the PE stationary with misordered columns (observed on HW).
        vaug = [pp.tile([128, KT, HD + 1], F16, tag=f"vaug{h}",
                        name=f"vaug{h}") for h in range(3)]
        onA = pp.tile([128, SEQ], F16, tag="onA")  # normalized out, heads 0/1
        onB = pp.tile([64, SEQ], F16, tag="onB")   # head 2
        woA = pp.tile([128, D], F16, tag="woA")
        woB = pp.tile([64, D], F16, tag="woB")
        w_sb = {n: [pp.tile([128, GD], F16, tag=f"w{n}{k}", name=f"w_{n}_{k}")
                    for k in range(6)] for n in ("q", "k", "v")}
        zt = pp.tile([128, QB], F16, tag="zt")    # zeros for PE warm-up

        nc.vector.memset(zt[:], 0.0)
        for h in range(3):
            nc.vector.memset(vaug[h][:, :, HD : HD + 1], 1.0)

        # weight DMAs issue on the scalar queue interleaved with its share of
        # x tiles (emitted inside issue_x below); wo lands late, on sync
        w_dma_todo = [(w_sb[name][k], wT[ds(k * 128, 128), :])
                      for name, wT in (("k", wkT), ("q", wqT), ("v", wvT))
                      for k in range(6)]

        xp = ctx.enter_context(tc.tile_pool(name="xp", bufs=12))
        mp = ctx.enter_context(tc.tile_pool(name="mp", bufs=16))
        # 6 P bufs = 2 full blocks of separation, so a new block's exp never
        # lands in a slot whose attV readers haven't been emitted yet
        pp2 = ctx.enter_context(tc.tile_pool(name="P", bufs=6))
        rp = ctx.enter_context(tc.tile_pool(name="rp", bufs=2))
        op = ctx.enter_context(tc.tile_pool(name="op", bufs=2))

        maskR = maskT.rearrange("(ko ki) q -> ki ko q", ki=128)
        masks = {}

        def issue_mask(n, j0=0, j1=8):
            tiles = masks.setdefault(n, [])
            for j in range(j0, j1):
                t = mp.tile([128, 2, QB], F16, tag="mask", name=f"mask{n}_{j}")
                eng = nc.gpsimd if j % 2 == 0 else nc.sync
                eng.dma_start(t[:], maskR[:, ds(j * 2, 2), ds(n * QB, QB)])
                tiles.append(t)

        # ---- phase 1: projections ----------------------------------------
        with tc.tile_pool(name="pj_ps", bufs=2, space="PSUM") as pj_ps, \
             tc.tile_pool(name="pv_ps", bufs=2, space="PSUM") as pv_ps:

            # PE warm-up: junk matmuls on zeros while the x DMAs stream in.
            # Keeps the HAM clock gate at 8/8 so the first real chains run
            # at 2.4 GHz (~12us of cover until xk lands).
            wps = pj_ps.tile([128, QB], F32, tag="warm")
            for _ in range(36):
                nc.tensor.matmul(wps[:], lhsT=zt[:, 0:128], rhs=zt[:],
                                 start=True, stop=True)

            # consumption order is k -> q -> v: energy needs the full kA at
            # attention step 0, q blocks and v tiles follow. x stripes over
            # all three DMA-capable queues (each queue serializes its
            # transfers at ~160GB/s); the w chunks ride along on scalar.
            x_engines = [nc.gpsimd, nc.sync, nc.scalar]

            def issue_x(name, xT):
                ts = []
                for nb2 in range(2):
                    for k in range(6):
                        xt = xp.tile([128, 1024], F16, tag="x",
                                     name=f"x_{name}_{nb2}_{k}")
                        eng = x_engines[(nb2 * 6 + k) % 3]
                        eng.dma_start(
                            xt[:], xT[ds(k * 128, 128), ds(nb2 * 1024, 1024)]
                        )
                        ts.append(xt)
                for _ in range(6):
                    if w_dma_todo:
                        wt, src = w_dma_todo.pop(0)
                        nc.scalar.dma_start(wt[:], src)
                return ts

            dests = {"q": (qA, qB), "k": (kA, kB)}
            xk_t = issue_x("k", xkT)
            for name in ("k", "q"):
                xs = xk_t if name == "k" else xq_t
                for nb2 in range(2):
                    for half in range(2):
                        n = nb2 * 2 + half
                        for mt in range(2):
                            mw = 128 if mt == 0 else 64
                            ps = pj_ps.tile([128, QB], F32, tag="pj")
                            for k in range(6):
                                nc.tensor.matmul(
                                    ps[0:mw, :],
                                    lhsT=w_sb[name][k][:, ds(mt * 128, mw)],
                                    rhs=xs[nb2 * 6 + k][:, ds(half * QB, QB)],
                                    start=(k == 0),
                                    stop=(k == 5),
                                )
                            dst = dests[name][mt]
                            nc.scalar.copy(
                                dst[0:mw, ds(n * QB, QB)], ps[0:mw, :]
                            )
                if name == "k":
                    xq_t = issue_x("q", xqT)
                else:
                    xv_t = issue_x("v", xvT)
                    issue_mask(0)   # after all x: masks only needed ~45us in
                    issue_mask(1)
                    nc.sync.dma_start(woA[:], woT[0:128, :])
                    nc.sync.dma_start(woB[:], woT[128:GD, :])
            for nb2 in range(2):
                for sb in range(8):
                    kt = nb2 * 8 + sb
                    pv = pv_ps.tile([128, 3, HD], F32, tag="pv")
                    for k in range(6):
                        nc.tensor.matmul(
                            pv[:, :, :],
                            lhsT=xv_t[nb2 * 6 + k][:, ds(sb * 128, 128)],
                            rhs=w_sb["v"][k][:].rearrange(
                                "p (h d) -> p h d", h=3
                            ),
                            start=(k == 0),
                            stop=(k == 5),
                        )
                    for h in range(3):
                        nc.vector.tensor_copy(
                            vaug[h][:, kt, 0:HD], pv[:, h, :]
                        )

        # ---- phase 2: attention + output projection, one pipeline -------
        q_of = (qA, qA, qB)
        k_of = (kA, kA, kB)
        pbase = (0, 64, 0)
        P = {}
        OU = {}
        pending = []   # deferred output-projection chains (n, j)

        with tc.tile_pool(name="e_ps", bufs=2, space="PSUM") as e_ps, \
             tc.tile_pool(name="ou_ps", bufs=3, space="PSUM") as ou_ps, \
             tc.tile_pool(name="f_ps", bufs=1, space="PSUM") as f_ps:

            def energy(n, g, h):
                e = e_ps.tile([128, 2, QB], F32, tag="e")
                p0 = pbase[h]
                for mm in range(2):
                    m = 2 * g + mm
                    nc.tensor.matmul(
                        e[:, mm, :],
                        lhsT=k_of[h][p0 : p0 + 64, ds(m * 128, 128)],
                        rhs=q_of[h][p0 : p0 + 64, ds(n * QB, QB)],
                        start=True,
                        stop=True,
                    )
                sl = ds(2 * g, 2)
                nc.scalar.activation(P[(n, h)][:, sl, :], e[:, :, :], Exp,
                                     scale=SCALE)
                nc.vector.tensor_mul(P[(n, h)][:, sl, :], P[(n, h)][:, sl, :],
                                     masks[n][g][:, :, :])

            def attv_seg(n, g):
                for h in range(3):
                    if g == 0:
                        OU[(n, h)] = ou_ps.tile([HD + 1, QB], F32, tag="ou",
                                                name=f"ou{n}_{h}")
                    ou = OU[(n, h)]
                    for mm in (2 * g, 2 * g + 1):
                        nc.tensor.matmul(
                            ou[:],
                            lhsT=vaug[h][:, mm, :],
                            rhs=P[(n, h)][:, mm, :],
                            start=(mm == 0),
                            stop=(mm == KT - 1),
                        )

            def norm_h(n, h):
                ou = OU.pop((n, h))
                # native tensor_copy remaps partition 64 -> 0; the custom
                # DVE recip op ignores partition offsets on its operands
                dsb = rp.tile([1, QB], F32, tag="dsb")
                nc.vector.tensor_copy(dsb[:], ou[HD : HD + 1, :])
                r1 = rp.tile([1, QB], F32, tag="r1")
                nc.vector.reciprocal_approx_fast(r1[:], dsb[:])
                rb = rp.tile([HD, QB], F32, tag="rb")
                nc.gpsimd.partition_broadcast(rb[:], r1[:])
                if h == 2:
                    dst = onB[0:HD, ds(n * QB, QB)]
                else:
                    dst = onA[pbase[h] : pbase[h] + HD, ds(n * QB, QB)]
                nc.vector.tensor_mul(dst, ou[0:HD, :], rb[:])

            osb = {}

            def outproj_chain(n, j, pool=None, tail=False):
                mq = n * 4 + j // 2
                half = j % 2
                c0 = half * 384
                if half == 0:
                    osb[mq] = op.tile([128, D], F16, tag="o", name=f"o{mq}")
                p = pool or f_ps
                f = p.tile([128, 384], F32, tag="ou" if p is ou_ps else "f")
                nc.tensor.matmul(f[:], lhsT=onA[:, ds(mq * 128, 128)],
                                 rhs=woA[:, ds(c0, 384)], start=True, stop=False)
                nc.tensor.matmul(f[:], lhsT=onB[0:HD, ds(mq * 128, 128)],
                                 rhs=woB[0:HD, ds(c0, 384)], start=False,
                                 stop=True)
                # in the tail, alternate copies over scalar+vector so neither
                # queue serializes all 16 of them; in-block, vector has slack
                if tail and half == 0:
                    nc.scalar.copy(osb[mq][:, ds(c0, 384)], f[:])
                else:
                    nc.vector.tensor_copy(osb[mq][:, ds(c0, 384)], f[:])
                if half == 1:
                    o = osb.pop(mq)
                    if tail:
                        # last block: 4-way split across 2 queues to shrink
                        # the final-store latency
                        for i in range(4):
                            eng = nc.sync if i % 2 == 0 else nc.scalar
                            eng.dma_start(
                                out[ds(mq * 128, 128), ds(i * 192, 192)],
                                o[:, ds(i * 192, 192)],
                            )
                    else:
                        nc.sync.dma_start(out[ds(mq * 128, 128), 0:384],
                                          o[:, 0:384])
                        nc.sync.dma_start(out[ds(mq * 128, 128), 384:D],
                                          o[:, 384:D])

            # attV lags energy by TWO steps: at a block boundary the new
            # block's first attV (which recycles the ou PSUM banks) lands a
            # full step after the previous block's norm reads
            LAG = 2
            TOT = NQB * NG
            for s in range(TOT + LAG):
                n, g = s // NG, s % NG
                if s < TOT:
                    if g == 0:
                        for h in range(3):
                            P[(n, h)] = pp2.tile([128, KT, QB], F16, tag="P",
                                                 name=f"P{n}_{h}")
                    energy(n, g, 0)
                    energy(n, g, 1)
                t = s - LAG
                if t >= 0 and t < TOT:
                    attv_seg(t // NG, t % NG)
                if s < TOT:
                    energy(n, g, 2)
                if t >= 0 and t % NG == NG - 1:
                    for h in range(3):
                        norm_h(t // NG, h)
                    pending.extend((t // NG, j) for j in range(8))
                if pending:
                    outproj_chain(*pending.pop(0))
                # mask prefetch for block n+2, only for groups whose block-n
                # readers (the mask muls) have already been emitted
                if s < TOT and n + 2 < NQB:
                    if g == 5:
                        issue_mask(n + 2, 0, 5)
                    elif g == 7:
                        issue_mask(n + 2, 5, 8)
            # tail: remaining output projection; ou banks are free now, so
            # rotate f tiles through them for a deeper PSUM pipeline
            tail_pools = [f_ps, ou_ps, ou_ps, ou_ps]
            ti = 0
            while pending:
                outproj_chain(*pending.pop(0), pool=tail_pools[ti % 4],
                              tail=True)
                ti += 1

        if debug:
            for nm, t in (("qA", qA), ("kA", kA), ("qB", qB), ("kB", kB),
                          ("onA", onA), ("onB", onB)):
                nc.sync.dma_start(dbg[nm][0 : t.shape[0], :], t[:])
            nc.sync.dma_start(
                dbg["vaug"][:, 0 : KT * (HD + 1)],
                vaug[0][:].rearrange("p a c -> p (a c)"),
            )

    nc.compile()
    return nc


def kernel(Q, K, V, mask, Wq, Wk, Wv, Wo):
    if "nc" not in _CACHE:
        _CACHE["nc"] = _build()
    nc = _CACHE["nc"]

    maskT_f16 = np.ascontiguousarray((mask[0, 0].T != 0).astype(np.float16))
    in_maps = []
    for c in range(8):
        b, g = c // 4, c % 4
        sl = slice(g * GD, (g + 1) * GD)
        in_maps.append(
            {
                "xqT": np.ascontiguousarray(Q[b].T.astype(np.float16)),
                "xkT": np.ascontiguousarray(K[b].T.astype(np.float16)),
                "xvT": np.ascontiguousarray(V[b].T.astype(np.float16)),
                "wqT": np.ascontiguousarray(Wq[sl, :].T.astype(np.float16)),
                "wkT": np.ascontiguousarray(Wk[sl, :].T.astype(np.float16)),
                "wvT": np.ascontiguousarray(Wv[sl, :].T.astype(np.float16)),
                "woT": np.ascontiguousarray(Wo[:, sl].T.astype(np.float16)),
                "maskT": maskT_f16,
            }
        )

    _install_profile_hook()
    res = run_bass_kernel_spmd(
        nc,
        in_maps,
        core_ids=list(range(8)),
        trace=bool(int(os.environ.get("KERNEL_PROFILE", "0"))),
    )
    _CACHE["last_exec_ns"] = res.exec_time_ns

    out = np.zeros((2, SEQ, D), dtype=np.float32)
    for c in range(8):
        out[c // 4] += res.results[c]["out"].astype(np.float32)
    return out


# revision 52
# speedup vs baseline: 1.0979x; 1.0730x over previous
"""Multi-head attention (B=2, S=2048, D=768, H=12) on 8 TRN2 NeuronCores.

Sharding: core c -> batch b = c//4, head-group g = c%4 (3 heads of 64 each).
Each core computes q/k/v projections for its 3 heads, masked softmax
attention, and a partial output projection against its 192 columns of Wo.
Host sums the 4 partial outputs per batch element (fp16 partials, fp32 sum).

Perf notes (v2):
  - fp16 everywhere on-device (same cost as bf16, more mantissa).
  - PE is HAM-clock-gated (1.2 GHz cold, 2.4 GHz after ~3.4us of sustained
    work): warm-up matmuls run during the initial DMA wait and the whole
    kernel is emitted as one gap-free tensor stream.
  - v is projected directly into [seq, d] layout (x-stationary matmuls) so
    no PE transposes are needed.
  - Attention is a 1-step-lagged pipeline over (n-block, k-group) steps:
    energy matmuls for group g run while exp/mask of g-1 and attV of g-1
    run on scalar/vector, keeping all engines busy.
  - Softmax denominators: ones-column in the v tiles -> row 64 of the attV
    PSUM; reciprocal via the fast custom-DVE approx on [1,512] (the plain
    reciprocal costs 3.3us), broadcast on gpsimd.
  - Output projection packs heads 0+1 into one K=128 matmul (onA) plus a
    K=64 accumulate (onB), interleaved one chain per pipeline step.
"""

import os
import sys

sys.path.insert(0, "/opt/trn_rl_repo")

from contextlib import ExitStack

import numpy as np

import concourse.bass as bass
import concourse.mybir as mybir
import concourse.tile as tile
from concourse import bacc
from concourse.bass import ds
from concourse.bass_utils import run_bass_kernel_spmd

F32 = mybir.dt.float32
F16 = mybir.dt.float16

SEQ = 2048
D = 768
HD = 64
GD = 192          # head-group width = 3 heads * 64
QB = 512          # q-block (free dim of E^T matmuls)
NQB = SEQ // QB   # 4
KT = SEQ // 128   # 16 k-tiles
NG = KT // 2      # 8 k-groups of 2 tiles per q-block
SCALE = float(1.0 / np.sqrt(np.float32(D)))

_CACHE = {}


def _install_profile_hook():
    """The image's antenv lacks axon_hooks; synthesize it so
    run_bass_kernel_spmd(trace=True) can reach the NTFF profiler in
    libaxon_pjrt.so (same ctypes shim trn_agent_boot uses)."""
    import types

    if "antenv.axon_hooks" in sys.modules:
        return
    sys.path.insert(0, "/root/.axon_site")
    try:
        from trn_agent_boot.trn_boot import _ntff_profile_via_ctypes
        hook = _ntff_profile_via_ctypes("/opt/axon/libaxon_pjrt.so")
    except Exception:
        hook = None
    import concourse.bass_utils as _bu

    _bu.upload_artifacts = lambda tmpdir: tmpdir  # no artifact bucket here
    mod = types.ModuleType("antenv.axon_hooks")
    mod.get_axon_ntff_profile_hook = lambda: hook
    mod.set_axon_ntff_profile_hook = lambda h: None
    sys.modules["antenv.axon_hooks"] = mod


def _build():
    nc = bacc.Bacc(None)

    xqT = nc.declare_dram_parameter("xqT", [D, SEQ], F16, isOutput=False)
    xkT = nc.declare_dram_parameter("xkT", [D, SEQ], F16, isOutput=False)
    xvT = nc.declare_dram_parameter("xvT", [D, SEQ], F16, isOutput=False)
    wqT = nc.declare_dram_parameter("wqT", [D, GD], F16, isOutput=False)
    wkT = nc.declare_dram_parameter("wkT", [D, GD], F16, isOutput=False)
    wvT = nc.declare_dram_parameter("wvT", [D, GD], F16, isOutput=False)
    woT = nc.declare_dram_parameter("woT", [GD, D], F16, isOutput=False)
    maskT = nc.declare_dram_parameter("maskT", [SEQ, SEQ], F16, isOutput=False)
    out = nc.declare_dram_parameter("out", [SEQ, D], F16, isOutput=True)
    debug = bool(int(os.environ.get("KERNEL_DEBUG", "0")))
    if debug:
        dbg = {
            nm: nc.declare_dram_parameter(f"dbg_{nm}", shp, F16, isOutput=True)
            for nm, shp in (
                ("qA", [128, SEQ]), ("kA", [128, SEQ]),
                ("qB", [64, SEQ]), ("kB", [64, SEQ]),
                ("onA", [128, SEQ]), ("onB", [64, SEQ]),
                ("vaug", [128, KT * 3 * (HD + 1)]),
            )
        }

    with tile.TileContext(nc) as tc, ExitStack() as ctx:
        Exp = mybir.ActivationFunctionType.Exp

        # ---- persistent tiles --------------------------------------------
        pp = ctx.enter_context(tc.tile_pool(name="persist", bufs=1))
        qA = pp.tile([128, SEQ], F16, tag="qA")   # heads 0 (p0-63) / 1 (p64-127)
        qB = pp.tile([64, SEQ], F16, tag="qB")    # head 2
        kA = pp.tile([128, SEQ], F16, tag="kA")
        kB = pp.tile([64, SEQ], F16, tag="kB")
        # v in [k-seq, d+1] layout per head; col 64 = ones (softmax denom).
        # Per-head 3D tiles: a 4D [128,KT,3,65] tile sliced [:,m,h,:] loads
        # the PE stationary with misordered columns (observed on HW).
        vaug = [pp.tile([128, KT, HD + 1], F16, tag=f"vaug{h}",
                        name=f"vaug{h}") for h in range(3)]
        onA = pp.tile([128, SEQ], F16, tag="onA")  # normalized out, heads 0/1
        onB = pp.tile([64, SEQ], F16, tag="onB")   # head 2
        woA = pp.tile([128, D], F16, tag="woA")
        woB = pp.tile([64, D], F16, tag="woB")
        w_sb = {n: [pp.tile([128, GD], F16, tag=f"w{n}{k}", name=f"w_{n}_{k}")
                    for k in range(6)] for n in ("q", "k", "v")}
        zt = pp.tile([128, QB], F16, tag="zt")    # zeros for PE warm-up

        nc.vector.memset(zt[:], 0.0)
        for h in range(3):
            nc.vector.memset(vaug[h][:, :, HD : HD + 1], 1.0)

        # weight DMAs issue on the scalar queue interleaved with its share of
        # x tiles (emitted inside issue_x below); wo lands late, on sync
        w_dma_todo = [(w_sb[name][k], wT[ds(k * 128, 128), :])
                      for name, wT in (("k", wkT), ("q", wqT), ("v", wvT))
                      for k in range(6)]

        xp = ctx.enter_context(tc.tile_pool(name="xp", bufs=12))
        mp = ctx.enter_context(tc.tile_pool(name="mp", bufs=16))
        # 6 P bufs = 2 full blocks of separation, so a new block's exp never
        # lands in a slot whose attV readers haven't been emitted yet
        pp2 = ctx.enter_context(tc.tile_pool(name="P", bufs=6))
        rp = ctx.enter_context(tc.tile_pool(name="rp", bufs=2))
        op = ctx.enter_context(tc.tile_pool(name="op", bufs=2))

        maskR = maskT.rearrange("(ko ki) q -> ki ko q", ki=128)
        masks = {}

        def issue_mask(n, j0=0, j1=8):
            tiles = masks.setdefault(n, [])
            for j in range(j0, j1):
                t = mp.tile([128, 2, QB], F16, tag="mask", name=f"mask{n}_{j}")
                eng = nc.gpsimd if j % 2 == 0 else nc.sync
                eng.dma_start(t[:], maskR[:, ds(j * 2, 2), ds(n * QB, QB)])
                tiles.append(t)

        # ---- phase 1: projections ----------------------------------------
        with tc.tile_pool(name="pj_ps", bufs=2, space="PSUM") as pj_ps, \
             tc.tile_pool(name="pv_ps", bufs=2, space="PSUM") as pv_ps:

            # PE warm-up: junk matmuls on zeros while the x DMAs stream in.
            # Keeps the HAM clock gate at 8/8 so the first real chains run
            # at 2.4 GHz (~12us of cover until xk lands).
            wps = pj_ps.tile([128, QB], F32, tag="warm")
            for _ in range(36):
                nc.tensor.matmul(wps[:], lhsT=zt[:, 0:128], rhs=zt[:],
                                 start=True, stop=True)

            # consumption order is k -> q -> v: energy needs the full kA at
            # attention step 0, q blocks and v tiles follow. x stripes over
            # all three DMA-capable queues (each queue serializes its
            # transfers at ~160GB/s); the w chunks ride along on scalar.
            x_engines = [nc.gpsimd, nc.sync, nc.scalar]

            def issue_x(name, xT):
                ts = []
                for nb2 in range(2):
                    for k in range(6):
                        xt = xp.tile([128, 1024], F16, tag="x",
                                     name=f"x_{name}_{nb2}_{k}")
                        eng = x_engines[(nb2 * 6 + k) % 3]
                        eng.dma_start(
                            xt[:], xT[ds(k * 128, 128), ds(nb2 * 1024, 1024)]
                        )
                        ts.append(xt)
                for _ in range(6):
                    if w_dma_todo:
                        wt, src = w_dma_todo.pop(0)
                        nc.scalar.dma_start(wt[:], src)
                return ts

            dests = {"q": (qA, qB), "k": (kA, kB)}
            xk_t = issue_x("k", xkT)
            for name in ("k", "q"):
                xs = xk_t if name == "k" else xq_t
                for nb2 in range(2):
                    for half in range(2):
                        n = nb2 * 2 + half
                        for mt in range(2):
                            mw = 128 if mt == 0 else 64
                            ps = pj_ps.tile([128, QB], F32, tag="pj")
                            for k in range(6):
                                nc.tensor.matmul(
                                    ps[0:mw, :],
                                    lhsT=w_sb[name][k][:, ds(mt * 128, mw)],
                                    rhs=xs[nb2 * 6 + k][:, ds(half * QB, QB)],
                                    start=(k == 0),
                                    stop=(k == 5),
                                )
                            dst = dests[name][mt]
                            nc.scalar.copy(
                                dst[0:mw, ds(n * QB, QB)], ps[0:mw, :]
                            )
                if name == "k":
                    xq_t = issue_x("q", xqT)
                else:
                    xv_t = issue_x("v", xvT)
                    issue_mask(0)   # after all x: masks only needed ~45us in
                    issue_mask(1)
                    nc.sync.dma_start(woA[:], woT[0:128, :])
                    nc.sync.dma_start(woB[:], woT[128:GD, :])
            pass
            # v-projection chains are deferred into the attention pipeline
            # (two per step over the first 8 steps) — they fill the PE's
            # idle slots while scalar exp paces the pipeline, keeping the
            # HAM clock gate warm, and attention starts ~11us earlier.

        # ---- phase 2: attention + output projection, one pipeline -------
        q_of = (qA, qA, qB)
        k_of = (kA, kA, kB)
        pbase = (0, 64, 0)
        P = {}
        OU = {}
        pending = []   # deferred output-projection chains (n, j)

        with tc.tile_pool(name="e_ps", bufs=2, space="PSUM") as e_ps, \
             tc.tile_pool(name="ou_ps", bufs=3, space="PSUM") as ou_ps, \
             tc.tile_pool(name="f_ps", bufs=1, space="PSUM") as f_ps:

            def pv_chain(kt):
                nb2, sb = kt // 8, kt % 8
                pv = e_ps.tile([128, 3, HD], F32, tag="e")
                for k in range(6):
                    nc.tensor.matmul(
                        pv[:, :, :],
                        lhsT=xv_t[nb2 * 6 + k][:, ds(sb * 128, 128)],
                        rhs=w_sb["v"][k][:].rearrange("p (h d) -> p h d", h=3),
                        start=(k == 0),
                        stop=(k == 5),
                    )
                for h in range(3):
                    nc.vector.tensor_copy(vaug[h][:, kt, 0:HD], pv[:, h, :])

            def energy(n, g, h):
                e = e_ps.tile([128, 2, QB], F32, tag="e")
                p0 = pbase[h]
                for mm in range(2):
                    m = 2 * g + mm
                    nc.tensor.matmul(
                        e[:, mm, :],
                        lhsT=k_of[h][p0 : p0 + 64, ds(m * 128, 128)],
                        rhs=q_of[h][p0 : p0 + 64, ds(n * QB, QB)],
                        start=True,
                        stop=True,
                    )
                sl = ds(2 * g, 2)
                nc.scalar.activation(P[(n, h)][:, sl, :], e[:, :, :], Exp,
                                     scale=SCALE)
                nc.vector.tensor_mul(P[(n, h)][:, sl, :], P[(n, h)][:, sl, :],
                                     masks[n][g][:, :, :])

            def attv_seg(n, g):
                for h in range(3):
                    if g == 0:
                        OU[(n, h)] = ou_ps.tile([HD + 1, QB], F32, tag="ou",
                                                name=f"ou{n}_{h}")
                    ou = OU[(n, h)]
                    for mm in (2 * g, 2 * g + 1):
                        nc.tensor.matmul(
                            ou[:],
                            lhsT=vaug[h][:, mm, :],
                            rhs=P[(n, h)][:, mm, :],
                            start=(mm == 0),
                            stop=(mm == KT - 1),
                        )

            def norm_h(n, h):
                ou = OU.pop((n, h))
                # native tensor_copy remaps partition 64 -> 0; the custom
                # DVE recip op ignores partition offsets on its operands
                dsb = rp.tile([1, QB], F32, tag="dsb")
                nc.vector.tensor_copy(dsb[:], ou[HD : HD + 1, :])
                r1 = rp.tile([1, QB], F32, tag="r1")
                nc.vector.reciprocal_approx_fast(r1[:], dsb[:])
                rb = rp.tile([HD, QB], F32, tag="rb")
                nc.gpsimd.partition_broadcast(rb[:], r1[:])
                if h == 2:
                    dst = onB[0:HD, ds(n * QB, QB)]
                else:
                    dst = onA[pbase[h] : pbase[h] + HD, ds(n * QB, QB)]
                nc.vector.tensor_mul(dst, ou[0:HD, :], rb[:])

            osb = {}

            def outproj_chain(n, j, pool=None, tail=False):
                mq = n * 4 + j // 2
                half = j % 2
                c0 = half * 384
                if half == 0:
                    osb[mq] = op.tile([128, D], F16, tag="o", name=f"o{mq}")
                p = pool or f_ps
                f = p.tile([128, 384], F32, tag="ou" if p is ou_ps else "f")
                nc.tensor.matmul(f[:], lhsT=onA[:, ds(mq * 128, 128)],
                                 rhs=woA[:, ds(c0, 384)], start=True, stop=False)
                nc.tensor.matmul(f[:], lhsT=onB[0:HD, ds(mq * 128, 128)],
                                 rhs=woB[0:HD, ds(c0, 384)], start=False,
                                 stop=True)
                # in the tail, alternate copies over scalar+vector so neither
                # queue serializes all 16 of them; in-block, vector has slack
                if tail and half == 0:
                    nc.scalar.copy(osb[mq][:, ds(c0, 384)], f[:])
                else:
                    nc.vector.tensor_copy(osb[mq][:, ds(c0, 384)], f[:])
                if half == 1:
                    o = osb.pop(mq)
                    if tail:
                        # last block: 4-way split across 2 queues to shrink
                        # the final-store latency
                        for i in range(4):
                            eng = nc.sync if i % 2 == 0 else nc.scalar
                            eng.dma_start(
                                out[ds(mq * 128, 128), ds(i * 192, 192)],
                                o[:, ds(i * 192, 192)],
                            )
                    else:
                        nc.sync.dma_start(out[ds(mq * 128, 128), 0:384],
                                          o[:, 0:384])
                        nc.sync.dma_start(out[ds(mq * 128, 128), 384:D],
                                          o[:, 384:D])

            # attV lags energy by TWO steps: at a block boundary the new
            # block's first attV (which recycles the ou PSUM banks) lands a
            # full step after the previous block's norm reads
            LAG = 2
            TOT = NQB * NG
            attv_done = set()

            def attv_try(t):
                if 0 <= t < TOT and t not in attv_done:
                    attv_done.add(t)
                    attv_seg(t // NG, t % NG)
                    if t % NG == NG - 1:
                        for h in range(3):
                            norm_h(t // NG, h)
                        pending.extend((t // NG, j) for j in range(8))

            for s in range(TOT + LAG):
                n, g = s // NG, s % NG
                if s < TOT:
                    if g == 0:
                        for h in range(3):
                            P[(n, h)] = pp2.tile([128, KT, QB], F16, tag="P",
                                                 name=f"P{n}_{h}")
                    energy(n, g, 0)
                    energy(n, g, 1)
                attv_try(s - LAG)
                if s < TOT:
                    energy(n, g, 2)
                if s < NG:
                    # v-projection: two chains per step over the first block
                    pv_chain(2 * s)
                    pv_chain(2 * s + 1)
                if s >= 30:
                    # collapse the last block's lag so the tail shrinks
                    attv_try(s - LAG + 1)
                if pending:
                    outproj_chain(*pending.pop(0))
                # mask prefetch for block n+2, only for groups whose block-n
                # readers (the mask muls) have already been emitted
                if s < TOT and n + 2 < NQB:
                    if g == 5:
                        issue_mask(n + 2, 0, 5)
                    elif g == 7:
                        issue_mask(n + 2, 5, 8)
            # tail: remaining output projection; ou banks are free now, so
            # rotate f tiles through them for a deeper PSUM pipeline
            tail_pools = [f_ps, ou_ps, ou_ps, ou_ps]
            ti = 0
            while pending:
                outproj_chain(*pending.pop(0), pool=tail_pools[ti % 4],
                              tail=True)
                ti += 1

        if debug:
            for nm, t in (("qA", qA), ("kA", kA), ("qB", qB), ("kB", kB),
                          ("onA", onA), ("onB", onB)):
                nc.sync.dma_start(dbg[nm][0 : t.shape[0], :], t[:])
            nc.sync.dma_start(
                dbg["vaug"][:, 0 : KT * (HD + 1)],
                vaug[0][:].rearrange("p a c -> p (a c)"),
            )

    nc.compile()
    return nc


def kernel(Q, K, V, mask, Wq, Wk, Wv, Wo):
    if "nc" not in _CACHE:
        _CACHE["nc"] = _build()
    nc = _CACHE["nc"]

    maskT_f16 = np.ascontiguousarray((mask[0, 0].T != 0).astype(np.float16))
    in_maps = []
    for c in range(8):
        b, g = c // 4, c % 4
        sl = slice(g * GD, (g + 1) * GD)
        in_maps.append(
            {
                "xqT": np.ascontiguousarray(Q[b].T.astype(np.float16)),
                "xkT": np.ascontiguousarray(K[b].T.astype(np.float16)),
                "xvT": np.ascontiguousarray(V[b].T.astype(np.float16)),
                "wqT": np.ascontiguousarray(Wq[sl, :].T.astype(np.float16)),
                "wkT": np.ascontiguousarray(Wk[sl, :].T.astype(np.float16)),
                "wvT": np.ascontiguousarray(Wv[sl, :].T.astype(np.float16)),
                "woT": np.ascontiguousarray(Wo[:, sl].T.astype(np.float16)),
                "maskT": maskT_f16,
            }
        )

    _install_profile_hook()
    res = run_bass_kernel_spmd(
        nc,
        in_maps,
        core_ids=list(range(8)),
        trace=bool(int(os.environ.get("KERNEL_PROFILE", "0"))),
    )
    _CACHE["last_exec_ns"] = res.exec_time_ns

    out = np.zeros((2, SEQ, D), dtype=np.float32)
    for c in range(8):
        out[c // 4] += res.results[c]["out"].astype(np.float32)
    return out


# revision 54
# speedup vs baseline: 1.2273x; 1.1178x over previous
"""Multi-head attention (B=2, S=2048, D=768, H=12) on 8 TRN2 NeuronCores.

Sharding: core c -> batch b = c//4, head-group g = c%4 (3 heads of 64 each).
Each core computes q/k/v projections for its 3 heads, masked softmax
attention, and a partial output projection against its 192 columns of Wo.
Host sums the 4 partial outputs per batch element (fp16 partials, fp32 sum).

Perf notes (v2):
  - fp16 everywhere on-device (same cost as bf16, more mantissa).
  - PE is HAM-clock-gated (1.2 GHz cold, 2.4 GHz after ~3.4us of sustained
    work): warm-up matmuls run during the initial DMA wait and the whole
    kernel is emitted as one gap-free tensor stream.
  - v is projected directly into [seq, d] layout (x-stationary matmuls) so
    no PE transposes are needed.
  - Attention is a 1-step-lagged pipeline over (n-block, k-group) steps:
    energy matmuls for group g run while exp/mask of g-1 and attV of g-1
    run on scalar/vector, keeping all engines busy.
  - Softmax denominators: ones-column in the v tiles -> row 64 of the attV
    PSUM; reciprocal via the fast custom-DVE approx on [1,512] (the plain
    reciprocal costs 3.3us), broadcast on gpsimd.
  - Output projection packs heads 0+1 into one K=128 matmul (onA) plus a
    K=64 accumulate (onB), interleaved one chain per pipeline step.
"""

import os
import sys

sys.path.insert(0, "/opt/trn_rl_repo")

from contextlib import ExitStack

import numpy as np

import concourse.bass as bass
import concourse.mybir as mybir
import concourse.tile as tile
from concourse import bacc
from concourse.bass import ds
from concourse.bass_utils import run_bass_kernel_spmd

F32 = mybir.dt.float32
F16 = mybir.dt.float16

SEQ = 2048
D = 768
HD = 64
GD = 192          # head-group width = 3 heads * 64
QB = 512          # q-block (free dim of E^T matmuls)
NQB = SEQ // QB   # 4
KT = SEQ // 128   # 16 k-tiles
NG = KT // 2      # 8 k-groups of 2 tiles per q-block
SCALE = float(1.0 / np.sqrt(np.float32(D)))

_CACHE = {}


def _install_profile_hook():
    """The image's antenv lacks axon_hooks; synthesize it so
    run_bass_kernel_spmd(trace=True) can reach the NTFF profiler in
    libaxon_pjrt.so (same ctypes shim trn_agent_boot uses)."""
    import types

    if "antenv.axon_hooks" in sys.modules:
        return
    sys.path.insert(0, "/root/.axon_site")
    try:
        from trn_agent_boot.trn_boot import _ntff_profile_via_ctypes
        hook = _ntff_profile_via_ctypes("/opt/axon/libaxon_pjrt.so")
    except Exception:
        hook = None
    import concourse.bass_utils as _bu

    _bu.upload_artifacts = lambda tmpdir: tmpdir  # no artifact bucket here
    mod = types.ModuleType("antenv.axon_hooks")
    mod.get_axon_ntff_profile_hook = lambda: hook
    mod.set_axon_ntff_profile_hook = lambda h: None
    sys.modules["antenv.axon_hooks"] = mod


def _build():
    nc = bacc.Bacc(None)

    xqT = nc.declare_dram_parameter("xqT", [D, SEQ], F16, isOutput=False)
    xkT = nc.declare_dram_parameter("xkT", [D, SEQ], F16, isOutput=False)
    xvT = nc.declare_dram_parameter("xvT", [D, SEQ], F16, isOutput=False)
    wqT = nc.declare_dram_parameter("wqT", [D, GD], F16, isOutput=False)
    wkT = nc.declare_dram_parameter("wkT", [D, GD], F16, isOutput=False)
    wvT = nc.declare_dram_parameter("wvT", [D, GD], F16, isOutput=False)
    woT = nc.declare_dram_parameter("woT", [GD, D], F16, isOutput=False)
    maskT = nc.declare_dram_parameter("maskT", [SEQ, SEQ], F16, isOutput=False)
    out = nc.declare_dram_parameter("out", [SEQ, D], F16, isOutput=True)
    debug = bool(int(os.environ.get("KERNEL_DEBUG", "0")))
    if debug:
        dbg = {
            nm: nc.declare_dram_parameter(f"dbg_{nm}", shp, F16, isOutput=True)
            for nm, shp in (
                ("qA", [128, SEQ]), ("kA", [128, SEQ]),
                ("qB", [64, SEQ]), ("kB", [64, SEQ]),
                ("onA", [128, SEQ]), ("onB", [64, SEQ]),
                ("vaug", [128, KT * 3 * (HD + 1)]),
            )
        }

    with tile.TileContext(nc) as tc, ExitStack() as ctx:
        Exp = mybir.ActivationFunctionType.Exp

        # ---- persistent tiles --------------------------------------------
        pp = ctx.enter_context(tc.tile_pool(name="persist", bufs=1))
        qA = pp.tile([128, SEQ], F16, tag="qA")   # heads 0 (p0-63) / 1 (p64-127)
        qB = pp.tile([64, SEQ], F16, tag="qB")    # head 2
        kA = pp.tile([128, SEQ], F16, tag="kA")
        kB = pp.tile([64, SEQ], F16, tag="kB")
        # v in [k-seq, d+1] layout per head; col 64 = ones (softmax denom).
        # Per-head 3D tiles: a 4D [128,KT,3,65] tile sliced [:,m,h,:] loads
        # the PE stationary with misordered columns (observed on HW).
        vaug = [pp.tile([128, KT, HD + 1], F16, tag=f"vaug{h}",
                        name=f"vaug{h}") for h in range(3)]
        onA = pp.tile([128, SEQ], F16, tag="onA")  # normalized out, heads 0/1
        onB = pp.tile([64, SEQ], F16, tag="onB")   # head 2
        woA = pp.tile([128, D], F16, tag="woA")
        woB = pp.tile([64, D], F16, tag="woB")
        w_sb = {n: [pp.tile([128, GD], F16, tag=f"w{n}{k}", name=f"w_{n}_{k}")
                    for k in range(6)] for n in ("q", "k", "v")}
        zt = pp.tile([128, QB], F16, tag="zt")    # zeros for PE warm-up

        nc.vector.memset(zt[:], 0.0)
        for h in range(3):
            nc.vector.memset(vaug[h][:, :, HD : HD + 1], 1.0)

        # weight DMAs issue on the scalar queue interleaved with its share of
        # x tiles (emitted inside issue_x below); wo lands late, on sync
        w_dma_todo = [(w_sb[name][k], wT[ds(k * 128, 128), :])
                      for name, wT in (("k", wkT), ("q", wqT), ("v", wvT))
                      for k in range(6)]

        xp = ctx.enter_context(tc.tile_pool(name="xp", bufs=12))
        mp = ctx.enter_context(tc.tile_pool(name="mp", bufs=16))
        # 6 P bufs = 2 full blocks of separation, so a new block's exp never
        # lands in a slot whose attV readers haven't been emitted yet
        pp2 = ctx.enter_context(tc.tile_pool(name="P", bufs=6))
        rp = ctx.enter_context(tc.tile_pool(name="rp", bufs=2))
        op = ctx.enter_context(tc.tile_pool(name="op", bufs=2))

        maskR = maskT.rearrange("(ko ki) q -> ki ko q", ki=128)
        masks = {}

        def issue_mask(n, j0=0, j1=8):
            tiles = masks.setdefault(n, [])
            for j in range(j0, j1):
                t = mp.tile([128, 2, QB], F16, tag="mask", name=f"mask{n}_{j}")
                eng = nc.gpsimd if j % 2 == 0 else nc.sync
                eng.dma_start(t[:], maskR[:, ds(j * 2, 2), ds(n * QB, QB)])
                tiles.append(t)

        # ---- phase 1: projections ----------------------------------------
        with tc.tile_pool(name="pj_ps", bufs=2, space="PSUM") as pj_ps, \
             tc.tile_pool(name="pv_ps", bufs=2, space="PSUM") as pv_ps:

            # PE warm-up: junk matmuls on zeros while the x DMAs stream in.
            # Keeps the HAM clock gate at 8/8 so the first real chains run
            # at 2.4 GHz (~12us of cover until xk lands).
            wps = pj_ps.tile([128, QB], F32, tag="warm")
            for _ in range(36):
                nc.tensor.matmul(wps[:], lhsT=zt[:, 0:128], rhs=zt[:],
                                 start=True, stop=True)

            # consumption order is k -> q -> v: energy needs the full kA at
            # attention step 0, q blocks and v tiles follow. x stripes over
            # all three DMA-capable queues (each queue serializes its
            # transfers at ~160GB/s); the w chunks ride along on scalar.
            x_engines = [nc.gpsimd, nc.sync, nc.scalar]

            def issue_x(name, xT):
                ts = []
                for nb2 in range(2):
                    for k in range(6):
                        xt = xp.tile([128, 1024], F16, tag="x",
                                     name=f"x_{name}_{nb2}_{k}")
                        eng = x_engines[(nb2 * 6 + k) % 3]
                        eng.dma_start(
                            xt[:], xT[ds(k * 128, 128), ds(nb2 * 1024, 1024)]
                        )
                        ts.append(xt)
                for _ in range(6):
                    if w_dma_todo:
                        wt, src = w_dma_todo.pop(0)
                        nc.scalar.dma_start(wt[:], src)
                return ts

            dests = {"q": (qA, qB), "k": (kA, kB)}
            xk_t = issue_x("k", xkT)
            for name in ("k", "q"):
                xs = xk_t if name == "k" else xq_t
                for nb2 in range(2):
                    for half in range(2):
                        n = nb2 * 2 + half
                        for mt in range(2):
                            mw = 128 if mt == 0 else 64
                            ps = pj_ps.tile([128, QB], F32, tag="pj")
                            for k in range(6):
                                nc.tensor.matmul(
                                    ps[0:mw, :],
                                    lhsT=w_sb[name][k][:, ds(mt * 128, mw)],
                                    rhs=xs[nb2 * 6 + k][:, ds(half * QB, QB)],
                                    start=(k == 0),
                                    stop=(k == 5),
                                )
                            dst = dests[name][mt]
                            nc.scalar.copy(
                                dst[0:mw, ds(n * QB, QB)], ps[0:mw, :]
                            )
                if name == "k":
                    xq_t = issue_x("q", xqT)
                else:
                    xv_t = issue_x("v", xvT)
                    issue_mask(0)   # after all x: masks only needed ~45us in
                    issue_mask(1)
                    nc.sync.dma_start(woA[:], woT[0:128, :])
                    nc.sync.dma_start(woB[:], woT[128:GD, :])
            for nb2 in range(2):
                for sb in range(8):
                    kt = nb2 * 8 + sb
                    pv = pv_ps.tile([128, 3, HD], F32, tag="pv")
                    for k in range(6):
                        nc.tensor.matmul(
                            pv[:, :, :],
                            lhsT=xv_t[nb2 * 6 + k][:, ds(sb * 128, 128)],
                            rhs=w_sb["v"][k][:].rearrange(
                                "p (h d) -> p h d", h=3
                            ),
                            start=(k == 0),
                            stop=(k == 5),
                        )
                    for h in range(3):
                        nc.vector.tensor_copy(
                            vaug[h][:, kt, 0:HD], pv[:, h, :]
                        )

        # ---- phase 2: attention + output projection, one pipeline -------
        q_of = (qA, qA, qB)
        k_of = (kA, kA, kB)
        pbase = (0, 64, 0)
        P = {}
        OU = {}
        pending = []   # deferred output-projection chains (n, j)

        with tc.tile_pool(name="e_ps", bufs=2, space="PSUM") as e_ps, \
             tc.tile_pool(name="ou_ps", bufs=3, space="PSUM") as ou_ps, \
             tc.tile_pool(name="f_ps", bufs=1, space="PSUM") as f_ps:

            def energy(n, g, h):
                e = e_ps.tile([128, 2, QB], F32, tag="e")
                p0 = pbase[h]
                for mm in range(2):
                    m = 2 * g + mm
                    nc.tensor.matmul(
                        e[:, mm, :],
                        lhsT=k_of[h][p0 : p0 + 64, ds(m * 128, 128)],
                        rhs=q_of[h][p0 : p0 + 64, ds(n * QB, QB)],
                        start=True,
                        stop=True,
                    )
                sl = ds(2 * g, 2)
                nc.scalar.activation(P[(n, h)][:, sl, :], e[:, :, :], Exp,
                                     scale=SCALE)
                nc.vector.tensor_mul(P[(n, h)][:, sl, :], P[(n, h)][:, sl, :],
                                     masks[n][g][:, :, :])

            def attv_seg(n, g):
                for h in range(3):
                    if g == 0:
                        OU[(n, h)] = ou_ps.tile([HD + 1, QB], F32, tag="ou",
                                                name=f"ou{n}_{h}")
                    ou = OU[(n, h)]
                    for mm in (2 * g, 2 * g + 1):
                        nc.tensor.matmul(
                            ou[:],
                            lhsT=vaug[h][:, mm, :],
                            rhs=P[(n, h)][:, mm, :],
                            start=(mm == 0),
                            stop=(mm == KT - 1),
                        )

            def norm_h(n, h):
                ou = OU.pop((n, h))
                # native tensor_copy remaps partition 64 -> 0; the custom
                # DVE recip op ignores partition offsets on its operands
                dsb = rp.tile([1, QB], F32, tag="dsb")
                nc.vector.tensor_copy(dsb[:], ou[HD : HD + 1, :])
                r1 = rp.tile([1, QB], F32, tag="r1")
                nc.vector.reciprocal_approx_fast(r1[:], dsb[:])
                rb = rp.tile([HD, QB], F32, tag="rb")
                nc.gpsimd.partition_broadcast(rb[:], r1[:])
                if h == 2:
                    dst = onB[0:HD, ds(n * QB, QB)]
                else:
                    dst = onA[pbase[h] : pbase[h] + HD, ds(n * QB, QB)]
                nc.vector.tensor_mul(dst, ou[0:HD, :], rb[:])

            osb = {}

            def outproj_chain(n, j, pool=None, tail=False):
                mq = n * 4 + j // 2
                half = j % 2
                c0 = half * 384
                if half == 0:
                    osb[mq] = op.tile([128, D], F16, tag="o", name=f"o{mq}")
                p = pool or f_ps
                f = p.tile([128, 384], F32, tag="ou" if p is ou_ps else "f")
                nc.tensor.matmul(f[:], lhsT=onA[:, ds(mq * 128, 128)],
                                 rhs=woA[:, ds(c0, 384)], start=True, stop=False)
                nc.tensor.matmul(f[:], lhsT=onB[0:HD, ds(mq * 128, 128)],
                                 rhs=woB[0:HD, ds(c0, 384)], start=False,
                                 stop=True)
                # in the tail, alternate copies over scalar+vector so neither
                # queue serializes all 16 of them; in-block, vector has slack
                if tail and half == 0:
                    nc.scalar.copy(osb[mq][:, ds(c0, 384)], f[:])
                else:
                    nc.vector.tensor_copy(osb[mq][:, ds(c0, 384)], f[:])
                if half == 1:
                    o = osb.pop(mq)
                    if tail:
                        # last block: 4-way split across 2 queues to shrink
                        # the final-store latency
                        for i in range(4):
                            eng = nc.sync if i % 2 == 0 else nc.scalar
                            eng.dma_start(
                                out[ds(mq * 128, 128), ds(i * 192, 192)],
                                o[:, ds(i * 192, 192)],
                            )
                    else:
                        nc.sync.dma_start(out[ds(mq * 128, 128), 0:384],
                                          o[:, 0:384])
                        nc.sync.dma_start(out[ds(mq * 128, 128), 384:D],
                                          o[:, 384:D])

            # attV lags energy by TWO steps: at a block boundary the new
            # block's first attV (which recycles the ou PSUM banks) lands a
            # full step after the previous block's norm reads
            LAG = 2
            TOT = NQB * NG
            for s in range(TOT + LAG + 1):
                n, g = s // NG, s % NG
                if s < TOT:
                    if g == 0:
                        for h in range(3):
                            P[(n, h)] = pp2.tile([128, KT, QB], F16, tag="P",
                                                 name=f"P{n}_{h}")
                    energy(n, g, 0)
                    energy(n, g, 1)
                t = s - LAG
                if t >= 1 and t % NG == 0:
                    # heads 1/2 of the previous block normalize here, right
                    # before the attV that recycles their ou banks — the
                    # boundary vector work is split across two steps instead
                    # of bunching in one
                    norm_h(t // NG - 1, 1)
                    norm_h(t // NG - 1, 2)
                    pending.extend((t // NG - 1, j) for j in range(8))
                if 0 <= t < TOT:
                    attv_seg(t // NG, t % NG)
                if s < TOT:
                    energy(n, g, 2)
                if t >= 0 and t % NG == NG - 1:
                    norm_h(t // NG, 0)
                if pending:
                    outproj_chain(*pending.pop(0))
                # mask prefetch for block n+2, only for groups whose block-n
                # readers (the mask muls) have already been emitted
                if s < TOT and n + 2 < NQB:
                    if g == 5:
                        issue_mask(n + 2, 0, 5)
                    elif g == 7:
                        issue_mask(n + 2, 5, 8)
            # tail: remaining output projection; ou banks are free now, so
            # rotate f tiles through them for a deeper PSUM pipeline
            tail_pools = [f_ps, ou_ps, ou_ps, ou_ps]
            ti = 0
            while pending:
                outproj_chain(*pending.pop(0), pool=tail_pools[ti % 4],
                              tail=True)
                ti += 1

        if debug:
            for nm, t in (("qA", qA), ("kA", kA), ("qB", qB), ("kB", kB),
                          ("onA", onA), ("onB", onB)):
                nc.sync.dma_start(dbg[nm][0 : t.shape[0], :], t[:])
            nc.sync.dma_start(
                dbg["vaug"][:, 0 : KT * (HD + 1)],
                vaug[0][:].rearrange("p a c -> p (a c)"),
            )

    nc.compile()
    return nc


def kernel(Q, K, V, mask, Wq, Wk, Wv, Wo):
    if "nc" not in _CACHE:
        _CACHE["nc"] = _build()
    nc = _CACHE["nc"]

    maskT_f16 = np.ascontiguousarray((mask[0, 0].T != 0).astype(np.float16))
    in_maps = []
    for c in range(8):
        b, g = c // 4, c % 4
        sl = slice(g * GD, (g + 1) * GD)
        in_maps.append(
            {
                "xqT": np.ascontiguousarray(Q[b].T.astype(np.float16)),
                "xkT": np.ascontiguousarray(K[b].T.astype(np.float16)),
                "xvT": np.ascontiguousarray(V[b].T.astype(np.float16)),
                "wqT": np.ascontiguousarray(Wq[sl, :].T.astype(np.float16)),
                "wkT": np.ascontiguousarray(Wk[sl, :].T.astype(np.float16)),
                "wvT": np.ascontiguousarray(Wv[sl, :].T.astype(np.float16)),
                "woT": np.ascontiguousarray(Wo[:, sl].T.astype(np.float16)),
                "maskT": maskT_f16,
            }
        )

    _install_profile_hook()
    res = run_bass_kernel_spmd(
        nc,
        in_maps,
        core_ids=list(range(8)),
        trace=bool(int(os.environ.get("KERNEL_PROFILE", "0"))),
    )
    _CACHE["last_exec_ns"] = res.exec_time_ns

    out = np.zeros((2, SEQ, D), dtype=np.float32)
    for c in range(8):
        out[c // 4] += res.results[c]["out"].astype(np.float32)
    return out
